# revision 10
# baseline (speedup 1.0000x reference)
"""Trainium2 Bass kernel for nn_BoxHead (FPN RoIAlign + 2-layer MLP + heads).

Strategy (8 NeuronCores, data-parallel over RoIs):
  - cores 0-3: image 0, 128 RoIs each; cores 4-7: image 1.
  - Host computes per-RoI FPN level + bilinear corner rows/weights, covers the
    needed x-points of each RoI row with 4-pixel-wide spans (4KB descriptors),
    and packs per-core index / interp-weight tensors. Feature maps are sent in
    HWC layout so a span is contiguous in DRAM.
  - Device: indirect-DMA gathers span tiles [128, 4*256]; per-RoI interp
    matmuls (contract subspans on partitions, accumulate the 4 x-offsets in
    PSUM) produce pooled features directly in channel-on-partition layout;
    then fp32 matmuls for 12544->1024->1024->(4|12) with W1 rows pre-permuted
    so no transposes are needed before layer 1.
  - SPMD: one program for all cores. Each core sorts its RoIs by size; the
    shared program is built for the slot-wise max size (order statistics agree
    closely across cores). Host un-permutes the outputs.
"""
import sys
import numpy as np

sys.path.insert(0, "/opt/trn_rl_repo")

P = 7
S = 2
IMG_W, IMG_H = 1088.0, 800.0
LEVEL_HW = [(200, 272), (100, 136), (50, 68), (25, 34)]
LEVEL_BASE = [0, 54400, 68000, 71400]
NPIX = 72250
NPIX_PAD = NPIX + 8
C = 256
XW = 2            # span width in pixels (2KB descriptors)
WXW = XW * P      # per-block wx width
WW = 7 + 2 * WXW  # packed weight row: ky | wxA | wxB
NCORES = 8
B, N = 2, 512
RPC = 128         # RoIs per core
GRID = ((np.arange(P, dtype=np.float32)[:, None]
         + ((np.arange(S, dtype=np.float32) + 0.5) / S)[None, :])
        .reshape(-1))
BIN = np.arange(P * S) // S  # sample -> bin

_PROGRAM_CACHE = {}
TRACE = False          # set True (with ntff shim installed) to profile
LAST_RESULT = None     # BassKernelResults of the last run


def _axis_prep(v, size):
    v = v.astype(np.float32)
    valid = (v >= -1.0) & (v <= size)
    vc = np.clip(v, 0.0, np.float32(size - 1.0)).astype(np.float32)
    lo = np.floor(vc).astype(np.float32)
    frac = (vc - lo).astype(np.float32)
    lo_i = lo.astype(np.int32)
    hi_i = np.minimum(lo_i + 1, size - 1).astype(np.int32)
    w0 = np.where(valid, (np.float32(1.0) - frac), np.float32(0.0)).astype(np.float32)
    w1 = np.where(valid, frac, np.float32(0.0)).astype(np.float32)
    return lo_i, hi_i, w0, w1


def _uniq_weights(lo, hi, w0, w1):
    """Unique corner coords + per-coord bin weights [nu, 7]."""
    u = np.unique(np.concatenate([lo, hi]))
    Wm = np.zeros((len(u), P), np.float32)
    np.add.at(Wm, (np.searchsorted(u, lo), BIN), w0)
    np.add.at(Wm, (np.searchsorted(u, hi), BIN), w1)
    return u, Wm


def _roi_prep(box, lvl):
    """-> (idx [nsub] int32 row indices, MT [nsub, XW, 49] fp32)."""
    H, W = LEVEL_HW[lvl]
    sx = np.float32(IMG_W / W)
    sy = np.float32(IMG_H / H)
    x1 = np.float32(box[0] / sx)
    y1 = np.float32(box[1] / sy)
    x2 = np.float32(box[2] / sx)
    y2 = np.float32(box[3] / sy)
    bw = np.float32(max(np.float32(x2 - x1), np.float32(1.0)) / np.float32(P))
    bh = np.float32(max(np.float32(y2 - y1), np.float32(1.0)) / np.float32(P))
    xs = (x1 + GRID * bw).astype(np.float32)
    ys = (y1 + GRID * bh).astype(np.float32)
    xlo, xhi, wx0, wx1 = _axis_prep(xs, W)
    ylo, yhi, wy0, wy1 = _axis_prep(ys, H)
    ux, Wx = _uniq_weights(xlo, xhi, wx0, wx1)
    uy, Wy = _uniq_weights(ylo, yhi, wy0, wy1)

    # greedy cover of ux with XW-wide spans
    starts = []
    i = 0
    while i < len(ux):
        s0 = int(ux[i])
        starts.append(s0)
        while i < len(ux) and ux[i] < s0 + XW:
            i += 1
    starts = np.array(starts, np.int64)
    nsp = len(starts)
    # per-span, per-offset x weights [nsp, XW, 7]
    WxS = np.zeros((nsp, XW, P), np.float32)
    cand = starts[:, None] + np.arange(XW)[None, :]        # [nsp, XW]
    pos = np.searchsorted(ux, cand)
    ok = (pos < len(ux))
    okpos = np.where(ok, pos, 0)
    hit = ok & (ux[okpos] == cand)
    WxS[hit] = Wx[okpos[hit]]

    idx = (LEVEL_BASE[lvl] + uy.astype(np.int64)[:, None] * W
           + starts[None, :]).reshape(-1).astype(np.int32)
    nu = len(uy)
    ky = np.broadcast_to((np.float32(0.25) * Wy)[:, None, :],
                         (nu, nsp, P)).reshape(nu * nsp, P)
    wx = np.broadcast_to(WxS.reshape(nsp, XW * P)[None, :, :],
                         (nu, nsp, XW * P)).reshape(nu * nsp, XW * P)
    return idx, np.ascontiguousarray(ky), np.ascontiguousarray(wx)


def _levels(boxes):
    w = boxes[:, 2] - boxes[:, 0]
    h = boxes[:, 3] - boxes[:, 1]
    return (np.clip(np.floor(4.0 + np.log(np.sqrt((w * h).astype(np.float32))
                                          / np.float32(224.0))), 2.0, 5.0)
            .astype(np.int32) - 2)


def _pad64(n):
    return max(64, (n + 63) // 64 * 64)


def _build_program(pair_sizes):
    """pair_sizes: tuple of 64 ints (multiples of 128). Returns (nc, NT)."""
    import concourse.bass as bass
    import concourse.mybir as mybir
    import concourse.tile as tile
    from concourse import bacc
    from concourse.masks import make_identity

    dt = mybir.dt
    f32 = dt.float32

    # pair structure: pair p = slots (2p, 2p+1); pair sizes are multiples of
    # 128 so chunks never straddle pairs.
    pair_chunks = []
    pos = 0
    for p in range(RPC // 2):
        L = pair_sizes[p]
        assert L % 128 == 0
        pair_chunks.append((pos // 128, L // 128))
        pos += L
    NT = pos // 128
    NCH = 98  # 12544 / 128

    nc = bacc.Bacc("TRN2", target_bir_lowering=False, debug=False,
                   num_devices=NCORES)
    feats = nc.dram_tensor("feats", [NPIX_PAD, C], f32, kind="ExternalInput")
    idx_d = nc.dram_tensor("idx", [128, NT], dt.int32, kind="ExternalInput")
    w_d = nc.dram_tensor("w", [NT, 128, WW], f32, kind="ExternalInput")
    w1_d = nc.dram_tensor("w1p", [12544, 1024], f32, kind="ExternalInput")
    w2_d = nc.dram_tensor("w2", [1024, 1024], f32, kind="ExternalInput")
    wh_d = nc.dram_tensor("wh", [1024, 16], f32, kind="ExternalInput")
    b1_d = nc.dram_tensor("b1", [1, 1024], f32, kind="ExternalInput")
    b2_d = nc.dram_tensor("b2", [1, 1024], f32, kind="ExternalInput")
    bh_d = nc.dram_tensor("bh", [1, 16], f32, kind="ExternalInput")
    out_d = nc.dram_tensor("out", [128, 16], f32, kind="ExternalOutput")

    with tile.TileContext(nc) as tc:
        with (
            tc.tile_pool(name="const", bufs=1) as cpool,
            tc.tile_pool(name="g", bufs=6) as gpool,
            tc.tile_pool(name="mtp", bufs=6) as mtpool,
            tc.tile_pool(name="wtp", bufs=6) as wpool,
            tc.tile_pool(name="w1", bufs=8) as w1pool,
            tc.tile_pool(name="sp", bufs=3) as spool,
            tc.tile_pool(name="pp", bufs=3, space="PSUM") as pairpool,
            tc.tile_pool(name="ps", bufs=2, space="PSUM") as pspool,
            tc.tile_pool(name="big", bufs=1, space="PSUM") as bigpool,
        ):
            idx_sb = cpool.tile([128, NT], dt.int32)
            nc.sync.dma_start(idx_sb[:], idx_d[:])
            w2_sb = cpool.tile([128, 8 * 1024], f32)
            nc.sync.dma_start(
                w2_sb[:].rearrange("p (j n) -> p j n", n=1024),
                w2_d.rearrange("(j p) n -> p j n", p=128))
            wh_sb = cpool.tile([128, 8 * 16], f32)
            nc.sync.dma_start(
                wh_sb[:].rearrange("p (j n) -> p j n", n=16),
                wh_d.rearrange("(j p) n -> p j n", p=128))
            b1_sb = cpool.tile([1, 1024], f32)
            nc.sync.dma_start(b1_sb[:], b1_d[:])
            b2_sb = cpool.tile([1, 1024], f32)
            nc.sync.dma_start(b2_sb[:], b2_d[:])
            bh_sb = cpool.tile([1, 16], f32)
            nc.sync.dma_start(bh_sb[:], bh_d[:])
            ones = cpool.tile([1, 128], f32)
            nc.gpsimd.memset(ones[:], 1.0)
            ident = cpool.tile([128, 128], f32)
            make_identity(nc, ident[:])

            # Y: pooled features, channel-on-partition: Y[h][c', s*128 + r]
            y0 = cpool.tile([128, 49 * 128], f32, tag="y0")
            y1 = cpool.tile([128, 49 * 128], f32, tag="y1")
            y_sb = [y0, y1]

            # ---- Phase 1: gather + interp ----
            ctx1 = nc.named_scope("interp"); ctx1.__enter__()
            g_tiles = {}
            mt_tiles = {}

            def ensure_tile(t):
                if t in g_tiles:
                    return
                g = gpool.tile([128, XW * C], f32, tag="g")
                nc.gpsimd.indirect_dma_start(
                    out=g[:], out_offset=None, in_=feats[:],
                    in_offset=bass.IndirectOffsetOnAxis(
                        ap=idx_sb[:, t:t + 1], axis=0),
                )
                wt = wpool.tile([128, WW], f32, tag="wt")
                nc.sync.dma_start(wt[:], w_d[t])
                m = mtpool.tile([128, XW * 98], f32, tag="mt")
                # expand MT[sub, xo, 98] = ky[sub, by] * wx[sub, ab, xo, bx]
                # (ab = block-diag column block; the unused block's wx is 0)
                mp = m[:].ap[0]
                wp = wt[:].ap[0]
                for ab in (0, 1):
                    out_ap = bass.AP(m[:].tensor, m[:].offset + ab * 49,
                                     [mp, [98, XW], [7, 7], [1, 7]])
                    ky_ap = bass.AP(wt[:].tensor, wt[:].offset,
                                    [wp, [0, XW], [1, 7], [0, 7]])
                    wx_ap = bass.AP(wt[:].tensor, wt[:].offset + 7 + WXW * ab,
                                    [wp, [7, XW], [0, 7], [1, 7]])
                    nc.vector.tensor_mul(out_ap, ky_ap, wx_ap)
                g_tiles[t] = g
                mt_tiles[t] = m

            for p in range(RPC // 2):
                c0, nch = pair_chunks[p]
                for t in range(c0, c0 + nch):
                    ensure_tile(t)
                pps = pairpool.tile([98, 256], f32, tag="pp")
                nmm = nch * XW
                mi = 0
                for t in range(c0, c0 + nch):
                    g, m = g_tiles[t], mt_tiles[t]
                    for xo in range(XW):
                        nc.tensor.matmul(
                            pps[:],
                            m[:, xo * 98:(xo + 1) * 98],
                            g[:, xo * C:(xo + 1) * C],
                            start=(mi == 0), stop=(mi == nmm - 1),
                        )
                        mi += 1
                sp = spool.tile([98, 256], f32, tag="sp")
                nc.scalar.copy(sp[:], pps[:])
                for h in (0, 1):
                    tr = pspool.tile([128, 98], f32, tag="ps")
                    nc.tensor.transpose(tr[:], sp[:, h * 128:(h + 1) * 128],
                                        ident[:98, :98])
                    for j, r in ((0, p), (1, RPC - 1 - p)):
                        nc.scalar.copy(
                            y_sb[h][:].rearrange("p (s r) -> p s r", r=128)[:, :, r],
                            tr[:, j * 49:(j + 1) * 49])

            ctx1.__exit__(None, None, None)
            # ---- Phase 2: layer 1 (fv @ W1p + b1, relu) ----
            ctx2 = nc.named_scope("mlp"); ctx2.__enter__()
            x1ps = bigpool.tile([128, 1024], f32, tag="big")
            for k in range(NCH):
                s, h = divmod(k, 2)
                w1t = w1pool.tile([128, 1024], f32, tag="w1")
                nc.sync.dma_start(w1t[:], w1_d[k * 128:(k + 1) * 128, :])
                for nh in (0, 1):
                    nc.tensor.matmul(
                        x1ps[:, nh * 512:(nh + 1) * 512],
                        y_sb[h][:, s * 128:(s + 1) * 128],
                        w1t[:, nh * 512:(nh + 1) * 512],
                        start=(k == 0), stop=False,
                    )
            for nh in (0, 1):
                nc.tensor.matmul(
                    x1ps[:, nh * 512:(nh + 1) * 512],
                    ones[:1, :], b1_sb[:1, nh * 512:(nh + 1) * 512],
                    start=False, stop=True,
                )
            x1s = cpool.tile([128, 1024], f32)
            nc.scalar.activation(x1s[:], x1ps[:],
                                 mybir.ActivationFunctionType.Relu)

            # ---- Phase 3: transpose x1 ----
            x1t = cpool.tile([128, 1024], f32)
            for j in range(8):
                tr = pspool.tile([128, 128], f32, tag="ps")
                nc.tensor.transpose(tr[:], x1s[:, j * 128:(j + 1) * 128], ident[:])
                nc.vector.tensor_copy(x1t[:, j * 128:(j + 1) * 128], tr[:])

            # ---- Phase 4: layer 2 ----
            x2ps = bigpool.tile([128, 1024], f32, tag="big")
            for j in range(8):
                for nh in (0, 1):
                    nc.tensor.matmul(
                        x2ps[:, nh * 512:(nh + 1) * 512],
                        x1t[:, j * 128:(j + 1) * 128],
                        w2_sb[:, j * 1024 + nh * 512: j * 1024 + (nh + 1) * 512],
                        start=(j == 0), stop=False,
                    )
            for nh in (0, 1):
                nc.tensor.matmul(
                    x2ps[:, nh * 512:(nh + 1) * 512],
                    ones[:1, :], b2_sb[:1, nh * 512:(nh + 1) * 512],
                    start=False, stop=True,
                )
            x2s = cpool.tile([128, 1024], f32)
            nc.scalar.activation(x2s[:], x2ps[:],
                                 mybir.ActivationFunctionType.Relu)

            # ---- Phase 5: transpose x2 ----
            x2t = cpool.tile([128, 1024], f32)
            for j in range(8):
                tr = pspool.tile([128, 128], f32, tag="ps")
                nc.tensor.transpose(tr[:], x2s[:, j * 128:(j + 1) * 128], ident[:])
                nc.vector.tensor_copy(x2t[:, j * 128:(j + 1) * 128], tr[:])

            # ---- Phase 6: heads ----
            hps = pspool.tile([128, 16], f32, tag="ps")
            for j in range(8):
                nc.tensor.matmul(
                    hps[:], x2t[:, j * 128:(j + 1) * 128],
                    wh_sb[:, j * 16:(j + 1) * 16],
                    start=(j == 0), stop=False,
                )
            nc.tensor.matmul(hps[:], ones[:1, :], bh_sb[:1, :],
                             start=False, stop=True)
            out_sb = cpool.tile([128, 16], f32)
            nc.scalar.copy(out_sb[:], hps[:])
            nc.sync.dma_start(out_d[:], out_sb[:])
            ctx2.__exit__(None, None, None)

    nc.compile()
    return nc, NT


def kernel(**inputs):
    from concourse.bass_utils import run_bass_kernel_spmd

    p_maps = [inputs["p2"], inputs["p3"], inputs["p4"], inputs["p5"]]
    proposals = np.asarray(inputs["proposals"], dtype=np.float32)
    boxes = proposals.reshape(-1, 4)
    lvls = _levels(boxes)

    # per-image HWC feats
    feats_img = []
    for b in range(B):
        parts = [np.ascontiguousarray(
            np.transpose(np.asarray(p_maps[l][b], dtype=np.float32), (1, 2, 0))
        ).reshape(-1, C) for l in range(4)]
        f = np.concatenate(parts + [np.zeros((NPIX_PAD - NPIX, C), np.float32)])
        feats_img.append(np.ascontiguousarray(f))

    # per-core RoI prep
    core_rois = []   # per core: list of (orig_slot_in_core, idx, ky, wx)
    for c in range(NCORES):
        img = c // 4
        r0 = img * N + (c % 4) * RPC
        rois = []
        for i in range(RPC):
            r = r0 + i
            idx, ky, wx = _roi_prep(boxes[r], int(lvls[r]))
            rois.append((i, idx, ky, wx))
        rois.sort(key=lambda x: -x[1].shape[0])
        core_rois.append(rois)

    # shared pair sizes: pair p = sorted slots (2p, 2p+1); size = max over
    # cores of the pair's total subspans, padded to a multiple of 128.
    def _pad128(n):
        return max(128, (n + 127) // 128 * 128)
    pair_sizes = tuple(
        _pad128(max(core_rois[c][p][1].shape[0]
                    + core_rois[c][RPC - 1 - p][1].shape[0]
                    for c in range(NCORES)))
        for p in range(RPC // 2))

    key = pair_sizes
    if key not in _PROGRAM_CACHE:
        _PROGRAM_CACHE[key] = _build_program(pair_sizes)
    nc, NT = _PROGRAM_CACHE[key]

    # W1 rows permuted to (s, h, c') order: row (2s+h)*128+c' = W1[(h*128+c')*49+s]
    W1 = np.asarray(inputs["W1"], dtype=np.float32)
    w1p = np.ascontiguousarray(
        W1.reshape(2, 128, 49, 1024).transpose(2, 0, 1, 3).reshape(12544, 1024))
    w2 = np.ascontiguousarray(np.asarray(inputs["W2"], dtype=np.float32))
    wh = np.ascontiguousarray(np.concatenate(
        [np.asarray(inputs["Wc"], dtype=np.float32),
         np.asarray(inputs["Wr"], dtype=np.float32)], axis=1))
    b1 = np.asarray(inputs["b1"], dtype=np.float32).reshape(1, 1024)
    b2 = np.asarray(inputs["b2"], dtype=np.float32).reshape(1, 1024)
    bh = np.concatenate([np.asarray(inputs["bc"], dtype=np.float32),
                         np.asarray(inputs["br"], dtype=np.float32)]).reshape(1, 16)

    in_maps = []
    perms = []
    for c in range(NCORES):
        idx_arr = np.zeros((NT * 128,), np.int32)
        w_arr = np.zeros((NT * 128, WW), np.float32)
        perm = np.zeros(RPC, np.int64)
        pos = 0
        for p in range(RPC // 2):
            at = pos
            for j, slot in ((0, p), (1, RPC - 1 - p)):
                orig_i, idx, ky, wx = core_rois[c][slot]
                n = idx.shape[0]
                idx_arr[at:at + n] = idx
                w_arr[at:at + n, 0:7] = ky
                w_arr[at:at + n, 7 + WXW * j:7 + WXW * (j + 1)] = wx
                perm[slot] = orig_i
                at += n
            pos += pair_sizes[p]
        perms.append(perm)
        w_arr = w_arr.reshape(NT, 128, WW)
        in_maps.append({
            "feats": feats_img[c // 4],
            "idx": np.ascontiguousarray(idx_arr.reshape(NT, 128).T),
            "w": w_arr,
            "w1p": w1p, "w2": w2, "wh": wh,
            "b1": b1, "b2": b2, "bh": bh,
        })

    res = run_bass_kernel_spmd(nc, in_maps, list(range(NCORES)), trace=TRACE)
    global LAST_RESULT
    LAST_RESULT = res

    logits = np.zeros((B * N, 4), np.float32)
    box_pred = np.zeros((B * N, 12), np.float32)
    for c in range(NCORES):
        o = res.results[c]["out"]  # [128 slots, 16]
        r0 = (c // 4) * N + (c % 4) * RPC
        rows = r0 + perms[c]
        logits[rows] = o[:, 0:4]
        box_pred[rows] = o[:, 4:16]
    return logits, box_pred


# revision 11
# speedup vs baseline: 1.1092x; 1.1092x over previous
"""Trainium2 Bass kernel for nn_BoxHead (FPN RoIAlign + 2-layer MLP + heads).

Strategy (8 NeuronCores, data-parallel over RoIs):
  - cores 0-3: image 0, 128 RoIs each; cores 4-7: image 1.
  - Host computes per-RoI FPN level + bilinear corner rows/weights, covers the
    needed x-points of each RoI row with 4-pixel-wide spans (4KB descriptors),
    and packs per-core index / interp-weight tensors. Feature maps are sent in
    HWC layout so a span is contiguous in DRAM.
  - Device: indirect-DMA gathers span tiles [128, 4*256]; per-RoI interp
    matmuls (contract subspans on partitions, accumulate the 4 x-offsets in
    PSUM) produce pooled features directly in channel-on-partition layout;
    then fp32 matmuls for 12544->1024->1024->(4|12) with W1 rows pre-permuted
    so no transposes are needed before layer 1.
  - SPMD: one program for all cores. Each core sorts its RoIs by size; the
    shared program is built for the slot-wise max size (order statistics agree
    closely across cores). Host un-permutes the outputs.
"""
import sys
import numpy as np

sys.path.insert(0, "/opt/trn_rl_repo")

P = 7
S = 2
IMG_W, IMG_H = 1088.0, 800.0
LEVEL_HW = [(200, 272), (100, 136), (50, 68), (25, 34)]
LEVEL_BASE = [0, 54400, 68000, 71400]
NPIX = 72250
NPIX_PAD = NPIX + 8
C = 256
XW = 4            # span width in pixels (4KB descriptors)
WXW = XW * P      # per-block wx width
WW = 7 + 2 * WXW  # packed weight row: ky | wxA | wxB
NCORES = 8
B, N = 2, 512
RPC = 128         # RoIs per core
GRID = ((np.arange(P, dtype=np.float32)[:, None]
         + ((np.arange(S, dtype=np.float32) + 0.5) / S)[None, :])
        .reshape(-1))
BIN = np.arange(P * S) // S  # sample -> bin

_PROGRAM_CACHE = {}
TRACE = False          # set True (with ntff shim installed) to profile
LAST_RESULT = None     # BassKernelResults of the last run


def _axis_prep(v, size):
    v = v.astype(np.float32)
    valid = (v >= -1.0) & (v <= size)
    vc = np.clip(v, 0.0, np.float32(size - 1.0)).astype(np.float32)
    lo = np.floor(vc).astype(np.float32)
    frac = (vc - lo).astype(np.float32)
    lo_i = lo.astype(np.int32)
    hi_i = np.minimum(lo_i + 1, size - 1).astype(np.int32)
    w0 = np.where(valid, (np.float32(1.0) - frac), np.float32(0.0)).astype(np.float32)
    w1 = np.where(valid, frac, np.float32(0.0)).astype(np.float32)
    return lo_i, hi_i, w0, w1


def _uniq_weights(lo, hi, w0, w1):
    """Unique corner coords + per-coord bin weights [nu, 7]."""
    u = np.unique(np.concatenate([lo, hi]))
    Wm = np.zeros((len(u), P), np.float32)
    np.add.at(Wm, (np.searchsorted(u, lo), BIN), w0)
    np.add.at(Wm, (np.searchsorted(u, hi), BIN), w1)
    return u, Wm


def _roi_prep(box, lvl):
    """-> (idx [nsub] int32 row indices, MT [nsub, XW, 49] fp32)."""
    H, W = LEVEL_HW[lvl]
    sx = np.float32(IMG_W / W)
    sy = np.float32(IMG_H / H)
    x1 = np.float32(box[0] / sx)
    y1 = np.float32(box[1] / sy)
    x2 = np.float32(box[2] / sx)
    y2 = np.float32(box[3] / sy)
    bw = np.float32(max(np.float32(x2 - x1), np.float32(1.0)) / np.float32(P))
    bh = np.float32(max(np.float32(y2 - y1), np.float32(1.0)) / np.float32(P))
    xs = (x1 + GRID * bw).astype(np.float32)
    ys = (y1 + GRID * bh).astype(np.float32)
    xlo, xhi, wx0, wx1 = _axis_prep(xs, W)
    ylo, yhi, wy0, wy1 = _axis_prep(ys, H)
    ux, Wx = _uniq_weights(xlo, xhi, wx0, wx1)
    uy, Wy = _uniq_weights(ylo, yhi, wy0, wy1)

    # greedy cover of ux with XW-wide spans
    starts = []
    i = 0
    while i < len(ux):
        s0 = int(ux[i])
        starts.append(s0)
        while i < len(ux) and ux[i] < s0 + XW:
            i += 1
    starts = np.array(starts, np.int64)
    nsp = len(starts)
    # per-span, per-offset x weights [nsp, XW, 7]
    WxS = np.zeros((nsp, XW, P), np.float32)
    cand = starts[:, None] + np.arange(XW)[None, :]        # [nsp, XW]
    pos = np.searchsorted(ux, cand)
    ok = (pos < len(ux))
    okpos = np.where(ok, pos, 0)
    hit = ok & (ux[okpos] == cand)
    WxS[hit] = Wx[okpos[hit]]

    idx = (LEVEL_BASE[lvl] + uy.astype(np.int64)[:, None] * W
           + starts[None, :]).reshape(-1).astype(np.int32)
    nu = len(uy)
    ky = np.broadcast_to((np.float32(0.25) * Wy)[:, None, :],
                         (nu, nsp, P)).reshape(nu * nsp, P)
    wx = np.broadcast_to(WxS.reshape(nsp, XW * P)[None, :, :],
                         (nu, nsp, XW * P)).reshape(nu * nsp, XW * P)
    return idx, np.ascontiguousarray(ky), np.ascontiguousarray(wx)


def _levels(boxes):
    w = boxes[:, 2] - boxes[:, 0]
    h = boxes[:, 3] - boxes[:, 1]
    return (np.clip(np.floor(4.0 + np.log(np.sqrt((w * h).astype(np.float32))
                                          / np.float32(224.0))), 2.0, 5.0)
            .astype(np.int32) - 2)


def _pad64(n):
    return max(64, (n + 63) // 64 * 64)


def _build_program(pair_sizes):
    """pair_sizes: tuple of 64 ints (multiples of 128). Returns (nc, NT)."""
    import concourse.bass as bass
    import concourse.mybir as mybir
    import concourse.tile as tile
    from concourse import bacc
    from concourse.masks import make_identity

    dt = mybir.dt
    f32 = dt.float32

    # pair structure: pair p = slots (2p, 2p+1); pair sizes are multiples of
    # 128 so chunks never straddle pairs.
    pair_chunks = []
    pos = 0
    for p in range(RPC // 2):
        L = pair_sizes[p]
        assert L % 128 == 0
        pair_chunks.append((pos // 128, L // 128))
        pos += L
    NT = pos // 128
    NCH = 98  # 12544 / 128

    nc = bacc.Bacc("TRN2", target_bir_lowering=False, debug=False,
                   num_devices=NCORES)
    feats = nc.dram_tensor("feats", [NPIX_PAD, C], f32, kind="ExternalInput")
    idx_d = nc.dram_tensor("idx", [128, NT], dt.int32, kind="ExternalInput")
    w_d = nc.dram_tensor("w", [NT, 128, WW], f32, kind="ExternalInput")
    w1_d = nc.dram_tensor("w1p", [12544, 1024], f32, kind="ExternalInput")
    w2_d = nc.dram_tensor("w2", [1024, 1024], f32, kind="ExternalInput")
    wh_d = nc.dram_tensor("wh", [1024, 16], f32, kind="ExternalInput")
    b1_d = nc.dram_tensor("b1", [1, 1024], f32, kind="ExternalInput")
    b2_d = nc.dram_tensor("b2", [1, 1024], f32, kind="ExternalInput")
    bh_d = nc.dram_tensor("bh", [1, 16], f32, kind="ExternalInput")
    out_d = nc.dram_tensor("out", [128, 16], f32, kind="ExternalOutput")

    with tile.TileContext(nc) as tc:
        with (
            tc.tile_pool(name="const", bufs=1) as cpool,
            tc.tile_pool(name="g", bufs=6) as gpool,
            tc.tile_pool(name="mtp", bufs=6) as mtpool,
            tc.tile_pool(name="wtp", bufs=6) as wpool,
            tc.tile_pool(name="w1", bufs=8) as w1pool,
            tc.tile_pool(name="sp", bufs=3) as spool,
            tc.tile_pool(name="pp", bufs=3, space="PSUM") as pairpool,
            tc.tile_pool(name="ps", bufs=2, space="PSUM") as pspool,
            tc.tile_pool(name="big", bufs=1, space="PSUM") as bigpool,
        ):
            idx_sb = cpool.tile([128, NT], dt.int32)
            nc.sync.dma_start(idx_sb[:], idx_d[:])
            w2_sb = cpool.tile([128, 8 * 1024], f32)
            nc.sync.dma_start(
                w2_sb[:].rearrange("p (j n) -> p j n", n=1024),
                w2_d.rearrange("(j p) n -> p j n", p=128))
            wh_sb = cpool.tile([128, 8 * 16], f32)
            nc.sync.dma_start(
                wh_sb[:].rearrange("p (j n) -> p j n", n=16),
                wh_d.rearrange("(j p) n -> p j n", p=128))
            b1_sb = cpool.tile([1, 1024], f32)
            nc.sync.dma_start(b1_sb[:], b1_d[:])
            b2_sb = cpool.tile([1, 1024], f32)
            nc.sync.dma_start(b2_sb[:], b2_d[:])
            bh_sb = cpool.tile([1, 16], f32)
            nc.sync.dma_start(bh_sb[:], bh_d[:])
            ones = cpool.tile([1, 128], f32)
            nc.gpsimd.memset(ones[:], 1.0)
            ident = cpool.tile([128, 128], f32)
            make_identity(nc, ident[:])

            # Y: pooled features, channel-on-partition: Y[h][c', s*128 + r]
            y0 = cpool.tile([128, 49 * 128], f32, tag="y0")
            y1 = cpool.tile([128, 49 * 128], f32, tag="y1")
            y_sb = [y0, y1]

            # ---- Phase 1: gather + interp ----
            ctx1 = nc.named_scope("interp"); ctx1.__enter__()
            g_tiles = {}
            mt_tiles = {}

            def ensure_tile(t):
                if t in g_tiles:
                    return
                g = gpool.tile([128, XW * C], f32, tag="g")
                nc.gpsimd.indirect_dma_start(
                    out=g[:], out_offset=None, in_=feats[:],
                    in_offset=bass.IndirectOffsetOnAxis(
                        ap=idx_sb[:, t:t + 1], axis=0),
                )
                wt = wpool.tile([128, WW], f32, tag="wt")
                nc.sync.dma_start(wt[:], w_d[t])
                m = mtpool.tile([128, XW * 98], f32, tag="mt")
                # expand MT[sub, xo, 98] = ky[sub, by] * wx[sub, ab, xo, bx]
                # (ab = block-diag column block; the unused block's wx is 0)
                mp = m[:].ap[0]
                wp = wt[:].ap[0]
                for ab in (0, 1):
                    out_ap = bass.AP(m[:].tensor, m[:].offset + ab * 49,
                                     [mp, [98, XW], [7, 7], [1, 7]])
                    ky_ap = bass.AP(wt[:].tensor, wt[:].offset,
                                    [wp, [0, XW], [1, 7], [0, 7]])
                    wx_ap = bass.AP(wt[:].tensor, wt[:].offset + 7 + WXW * ab,
                                    [wp, [7, XW], [0, 7], [1, 7]])
                    nc.vector.tensor_mul(out_ap, ky_ap, wx_ap)
                g_tiles[t] = g
                mt_tiles[t] = m

            for p in range(RPC // 2):
                c0, nch = pair_chunks[p]
                for t in range(c0, c0 + nch):
                    ensure_tile(t)
                pps = pairpool.tile([98, 256], f32, tag="pp")
                nmm = nch * XW
                mi = 0
                for t in range(c0, c0 + nch):
                    g, m = g_tiles[t], mt_tiles[t]
                    for xo in range(XW):
                        nc.tensor.matmul(
                            pps[:],
                            m[:, xo * 98:(xo + 1) * 98],
                            g[:, xo * C:(xo + 1) * C],
                            start=(mi == 0), stop=(mi == nmm - 1),
                        )
                        mi += 1
                sp = spool.tile([98, 256], f32, tag="sp")
                nc.scalar.copy(sp[:], pps[:])
                for h in (0, 1):
                    tr = pspool.tile([128, 98], f32, tag="ps")
                    nc.tensor.transpose(tr[:], sp[:, h * 128:(h + 1) * 128],
                                        ident[:98, :98])
                    for j, r in ((0, p), (1, RPC - 1 - p)):
                        nc.scalar.copy(
                            y_sb[h][:].rearrange("p (s r) -> p s r", r=128)[:, :, r],
                            tr[:, j * 49:(j + 1) * 49])

            ctx1.__exit__(None, None, None)
            # ---- Phase 2: layer 1 (fv @ W1p + b1, relu) ----
            ctx2 = nc.named_scope("mlp"); ctx2.__enter__()
            x1ps = bigpool.tile([128, 1024], f32, tag="big")
            for k in range(NCH):
                s, h = divmod(k, 2)
                w1t = w1pool.tile([128, 1024], f32, tag="w1")
                nc.sync.dma_start(w1t[:], w1_d[k * 128:(k + 1) * 128, :])
                for nh in (0, 1):
                    nc.tensor.matmul(
                        x1ps[:, nh * 512:(nh + 1) * 512],
                        y_sb[h][:, s * 128:(s + 1) * 128],
                        w1t[:, nh * 512:(nh + 1) * 512],
                        start=(k == 0), stop=False,
                    )
            for nh in (0, 1):
                nc.tensor.matmul(
                    x1ps[:, nh * 512:(nh + 1) * 512],
                    ones[:1, :], b1_sb[:1, nh * 512:(nh + 1) * 512],
                    start=False, stop=True,
                )
            x1s = cpool.tile([128, 1024], f32)
            nc.scalar.activation(x1s[:], x1ps[:],
                                 mybir.ActivationFunctionType.Relu)

            # ---- Phase 3: transpose x1 ----
            x1t = cpool.tile([128, 1024], f32)
            for j in range(8):
                tr = pspool.tile([128, 128], f32, tag="ps")
                nc.tensor.transpose(tr[:], x1s[:, j * 128:(j + 1) * 128], ident[:])
                nc.vector.tensor_copy(x1t[:, j * 128:(j + 1) * 128], tr[:])

            # ---- Phase 4: layer 2 ----
            x2ps = bigpool.tile([128, 1024], f32, tag="big")
            for j in range(8):
                for nh in (0, 1):
                    nc.tensor.matmul(
                        x2ps[:, nh * 512:(nh + 1) * 512],
                        x1t[:, j * 128:(j + 1) * 128],
                        w2_sb[:, j * 1024 + nh * 512: j * 1024 + (nh + 1) * 512],
                        start=(j == 0), stop=False,
                    )
            for nh in (0, 1):
                nc.tensor.matmul(
                    x2ps[:, nh * 512:(nh + 1) * 512],
                    ones[:1, :], b2_sb[:1, nh * 512:(nh + 1) * 512],
                    start=False, stop=True,
                )
            x2s = cpool.tile([128, 1024], f32)
            nc.scalar.activation(x2s[:], x2ps[:],
                                 mybir.ActivationFunctionType.Relu)

            # ---- Phase 5: transpose x2 ----
            x2t = cpool.tile([128, 1024], f32)
            for j in range(8):
                tr = pspool.tile([128, 128], f32, tag="ps")
                nc.tensor.transpose(tr[:], x2s[:, j * 128:(j + 1) * 128], ident[:])
                nc.vector.tensor_copy(x2t[:, j * 128:(j + 1) * 128], tr[:])

            # ---- Phase 6: heads ----
            hps = pspool.tile([128, 16], f32, tag="ps")
            for j in range(8):
                nc.tensor.matmul(
                    hps[:], x2t[:, j * 128:(j + 1) * 128],
                    wh_sb[:, j * 16:(j + 1) * 16],
                    start=(j == 0), stop=False,
                )
            nc.tensor.matmul(hps[:], ones[:1, :], bh_sb[:1, :],
                             start=False, stop=True)
            out_sb = cpool.tile([128, 16], f32)
            nc.scalar.copy(out_sb[:], hps[:])
            nc.sync.dma_start(out_d[:], out_sb[:])
            ctx2.__exit__(None, None, None)

    nc.compile()
    return nc, NT


def kernel(**inputs):
    from concourse.bass_utils import run_bass_kernel_spmd

    p_maps = [inputs["p2"], inputs["p3"], inputs["p4"], inputs["p5"]]
    proposals = np.asarray(inputs["proposals"], dtype=np.float32)
    boxes = proposals.reshape(-1, 4)
    lvls = _levels(boxes)

    # per-image HWC feats
    feats_img = []
    for b in range(B):
        parts = [np.ascontiguousarray(
            np.transpose(np.asarray(p_maps[l][b], dtype=np.float32), (1, 2, 0))
        ).reshape(-1, C) for l in range(4)]
        f = np.concatenate(parts + [np.zeros((NPIX_PAD - NPIX, C), np.float32)])
        feats_img.append(np.ascontiguousarray(f))

    # per-core RoI prep
    core_rois = []   # per core: list of (orig_slot_in_core, idx, ky, wx)
    for c in range(NCORES):
        img = c // 4
        r0 = img * N + (c % 4) * RPC
        rois = []
        for i in range(RPC):
            r = r0 + i
            idx, ky, wx = _roi_prep(boxes[r], int(lvls[r]))
            rois.append((i, idx, ky, wx))
        rois.sort(key=lambda x: -x[1].shape[0])
        core_rois.append(rois)

    # shared pair sizes: pair p = sorted slots (2p, 2p+1); size = max over
    # cores of the pair's total subspans, padded to a multiple of 128.
    def _pad128(n):
        return max(128, (n + 127) // 128 * 128)
    pair_sizes = tuple(
        _pad128(max(core_rois[c][p][1].shape[0]
                    + core_rois[c][RPC - 1 - p][1].shape[0]
                    for c in range(NCORES)))
        for p in range(RPC // 2))

    key = pair_sizes
    if key not in _PROGRAM_CACHE:
        _PROGRAM_CACHE[key] = _build_program(pair_sizes)
    nc, NT = _PROGRAM_CACHE[key]

    # W1 rows permuted to (s, h, c') order: row (2s+h)*128+c' = W1[(h*128+c')*49+s]
    W1 = np.asarray(inputs["W1"], dtype=np.float32)
    w1p = np.ascontiguousarray(
        W1.reshape(2, 128, 49, 1024).transpose(2, 0, 1, 3).reshape(12544, 1024))
    w2 = np.ascontiguousarray(np.asarray(inputs["W2"], dtype=np.float32))
    wh = np.ascontiguousarray(np.concatenate(
        [np.asarray(inputs["Wc"], dtype=np.float32),
         np.asarray(inputs["Wr"], dtype=np.float32)], axis=1))
    b1 = np.asarray(inputs["b1"], dtype=np.float32).reshape(1, 1024)
    b2 = np.asarray(inputs["b2"], dtype=np.float32).reshape(1, 1024)
    bh = np.concatenate([np.asarray(inputs["bc"], dtype=np.float32),
                         np.asarray(inputs["br"], dtype=np.float32)]).reshape(1, 16)

    in_maps = []
    perms = []
    for c in range(NCORES):
        idx_arr = np.zeros((NT * 128,), np.int32)
        w_arr = np.zeros((NT * 128, WW), np.float32)
        perm = np.zeros(RPC, np.int64)
        pos = 0
        for p in range(RPC // 2):
            at = pos
            for j, slot in ((0, p), (1, RPC - 1 - p)):
                orig_i, idx, ky, wx = core_rois[c][slot]
                n = idx.shape[0]
                idx_arr[at:at + n] = idx
                w_arr[at:at + n, 0:7] = ky
                w_arr[at:at + n, 7 + WXW * j:7 + WXW * (j + 1)] = wx
                perm[slot] = orig_i
                at += n
            pos += pair_sizes[p]
        perms.append(perm)
        w_arr = w_arr.reshape(NT, 128, WW)
        in_maps.append({
            "feats": feats_img[c // 4],
            "idx": np.ascontiguousarray(idx_arr.reshape(NT, 128).T),
            "w": w_arr,
            "w1p": w1p, "w2": w2, "wh": wh,
            "b1": b1, "b2": b2, "bh": bh,
        })

    res = run_bass_kernel_spmd(nc, in_maps, list(range(NCORES)), trace=TRACE)
    global LAST_RESULT
    LAST_RESULT = res

    logits = np.zeros((B * N, 4), np.float32)
    box_pred = np.zeros((B * N, 12), np.float32)
    for c in range(NCORES):
        o = res.results[c]["out"]  # [128 slots, 16]
        r0 = (c // 4) * N + (c % 4) * RPC
        rows = r0 + perms[c]
        logits[rows] = o[:, 0:4]
        box_pred[rows] = o[:, 4:16]
    return logits, box_pred
